# revision 21
# baseline (speedup 1.0000x reference)
"""Trainium2 Bass kernel for CausalPriorityAttention.

Data-parallel over the batch dim: core b computes batch b (B=8, 8 cores).

Per-core dataflow (v2 — pipelined, single ACT table set):
  phase 1: qkT = (W_qk @ x^T + b)*scale  (Q^T,K^T in [feat, seq], fp16)
           V   = x @ W_v^T + vb          ([seq, feat] + ones col, fp16)
           E   = exp(a*sigmoid(gb)) via tanh identity: sigmoid(x) =
                 0.5*tanh(x/2)+0.5, so tanh+exp share ONE ACT table set.
  phase 2 (per (qc, head-pair) group, kt-slab pipelined):
           sT[k,q]  = K @ Q'^T  (row-group-paired K=64 matmuls -> PSUM)
           es       = exp(a*sT - 5)            (ACT, one 1024-wide op)
           eT       = es * E[kt]               (DVE fp16 2x, E repeated via
                                                stride-0 AP across both subs)
           pvab     += [V_h|1]^T @ eT          (accumulated over kt slabs)
           attnT    = pvab[0:64] * recip(pvab[64])  (per-group, merged recip)
  phase 3: y = attnT^T @ Wo^T + bo   (per seq tile; qc0 tiles start early)
Q' is prescaled by 1/(8*a) so exp's scale=a restores QK/8; the -5 shift
cancels in normalization and keeps exp products in fp16 range.
Weights (wT, woT) are loaded once outside the rep loop; per-rep DMAs (x,
graph_bias) overlap the previous rep's compute since their landing tiles
free mid-rep (tanh runs in-place on the gb tile).
"""

import sys

for _p in ("/opt/trn_rl_repo",):
    if _p not in sys.path:
        sys.path.append(_p)

import numpy as np

import concourse.bacc as bacc
import concourse.bass as bass
import concourse.mybir as mybir
import concourse.tile as tile
from concourse.bass_utils import run_bass_kernel_spmd

B, N, D = 8, 1024, 512
H, HD = 8, 64
P = 128
NT = N // P          # 8 seq tiles
KT = D // P          # 4 contraction tiles over D
FT_QK = 2 * D // P   # 8 feature tiles over [Q;K]
QC = N // 512        # 2 q-chunks of 512
F32 = mybir.dt.float32
F32R = mybir.dt.float32r
F16 = mybir.dt.float16

_CACHE = {}


def build_nc(ten_a: float, reps: int = 1):
    nc = bacc.Bacc("TRN2")
    xT = nc.dram_tensor("xT", [D, N], F32R, kind="ExternalInput")
    wT = nc.dram_tensor("wT", [D, 3 * D], F32R, kind="ExternalInput")
    woT = nc.dram_tensor("woT", [D, D], F32R, kind="ExternalInput")
    gb = nc.dram_tensor("gb", [N, N], F32, kind="ExternalInput")
    qkb = nc.dram_tensor("qkb", [P, FT_QK], F32, kind="ExternalInput")
    vb = nc.dram_tensor("vb", [D], F32, kind="ExternalInput")
    bo = nc.dram_tensor("bo", [1, D], F32R, kind="ExternalInput")
    ones1 = nc.dram_tensor("ones1", [1, P], F32R, kind="ExternalInput")
    y = nc.dram_tensor("y", [N, D], F32, kind="ExternalOutput")

    sQ = 1.0 / (8.0 * ten_a)
    Exp = mybir.ActivationFunctionType.Exp

    with tile.TileContext(nc) as tc:
        with tc.tile_pool(name="const", bufs=1) as const_pool, \
             tc.tile_pool(name="persist", bufs=1) as persist, \
             tc.tile_pool(name="io", bufs=1) as io, \
             tc.tile_pool(name="work", bufs=1) as work, \
             tc.tile_pool(name="ps_g", bufs=2, space="PSUM") as ps_g, \
             tc.tile_pool(name="ps_s", bufs=2, space="PSUM") as ps_s, \
             tc.tile_pool(name="ps_pv", bufs=1, space="PSUM") as ps_pv:
            # ---- persistent constants / weights (loaded once) ----
            qkb_sb = const_pool.tile([P, FT_QK], F32)
            nc.sync.dma_start(out=qkb_sb, in_=qkb[:, :])
            vb_sb = const_pool.tile([P, D], F32)
            nc.sync.dma_start(
                out=vb_sb,
                in_=bass.AP(tensor=vb.ap().tensor, offset=0, ap=[[0, P], [1, D]]),
            )
            bo_sb = const_pool.tile([1, D], F32R)
            nc.sync.dma_start(out=bo_sb, in_=bo[:, :])
            ones1_sb = const_pool.tile([1, P], F32R)
            nc.sync.dma_start(out=ones1_sb, in_=ones1[:, :])
            neg5 = const_pool.tile([P, 1], F32)
            nc.vector.memset(neg5, -5.0)
            b05 = const_pool.tile([P, 1], F32)
            nc.vector.memset(b05, 0.5 * ten_a)
            wT_sb = const_pool.tile([P, KT, 3 * D], F32R)
            nc.sync.dma_start(
                out=wT_sb, in_=wT[:, :].rearrange("(t p) n -> p t n", p=P)
            )
            woT_sb = const_pool.tile([P, KT, D], F32R)
            nc.sync.dma_start(
                out=woT_sb, in_=woT[:, :].rearrange("(t p) n -> p t n", p=P)
            )

            # ---- persistent per-rep tensors (single-buffered; WAR deps
            # between reps are tracked per-subtile) ----
            qkT = persist.tile([P, FT_QK, N], F16)
            v_sb = persist.tile([P, NT, H, HD + 1], F16)
            nc.vector.memset(v_sb[:, :, :, HD : HD + 1], 1.0)
            attnT = persist.tile([P, KT, N], F32R)

            # phase-1 group order: pair-0's K (ft4) and Q (ft0) first, then
            # all V seq tiles (PV consumes them slab by slab), then the rest.
            groups_a = [("qk", 4, 0), ("qk", 4, 1), ("qk", 0, 0), ("qk", 0, 1)]
            groups_b = [("v", st, 0) for st in range(NT)]
            for hp in range(1, 4):
                groups_b += [
                    ("qk", 4 + hp, 0), ("qk", 4 + hp, 1),
                    ("qk", hp, 0), ("qk", hp, 1),
                ]

            def phase3(st):
                yp = ps_g.tile([P, D], F32, tag="g", bufs=2, name="yp")
                for ft in range(KT):
                    nc.tensor.matmul(
                        yp,
                        lhsT=attnT[:, ft, st * P : (st + 1) * P],
                        rhs=woT_sb[:, ft, :],
                        start=(ft == 0),
                        stop=False,
                    )
                # rank-1 ones-row matmul adds bo into the psum
                nc.tensor.matmul(
                    yp, lhsT=ones1_sb, rhs=bo_sb, start=False, stop=True
                )
                ysb = work.tile([P, D], F32, tag="ysb", bufs=2, name="ysb")
                nc.vector.tensor_copy(ysb, yp)
                nc.sync.dma_start(out=y[st * P : (st + 1) * P, :], in_=ysb)

            def input_dmas():
                """Emit the x / graph_bias DMAs for one rep; returns tiles.
                Emitted mid-previous-rep so the transfers overlap compute
                (WAR deps gate the actual start)."""
                xT_sb = io.tile([P, KT, N], F32R, tag="xT", bufs=1,
                                name="xT_sb")
                for k in range(KT):
                    nc.sync.dma_start(
                        out=xT_sb[:, k, :], in_=xT[k * P : (k + 1) * P, :]
                    )
                gbt = io.tile([P, NT, N], F32, tag="gbt", bufs=1, name="gbt")
                for t4 in range(4):
                    nc.sync.dma_start(
                        out=gbt[:, 2 * t4 : 2 * t4 + 2, :],
                        in_=gb[256 * t4 : 256 * (t4 + 1), :].rearrange(
                            "(t p) n -> p t n", p=P
                        ),
                    )
                return xT_sb, gbt

            def proj_groups(blocks, xT_sb):
                for b0 in range(0, len(blocks), 2):
                    blk = blocks[b0 : b0 + 2]
                    tiles = [
                        ps_g.tile([P, D], F32, tag="g", bufs=2, name="g")
                        for _ in blk
                    ]
                    for k in range(KT):
                        for g, t in zip(blk, tiles):
                            if g[0] == "qk":
                                _, ft, qc = g
                                nc.tensor.matmul(
                                    t,
                                    lhsT=wT_sb[:, k, ft * P : (ft + 1) * P],
                                    rhs=xT_sb[:, k, qc * 512 : (qc + 1) * 512],
                                    start=(k == 0),
                                    stop=(k == KT - 1),
                                )
                            else:
                                _, st, _ = g
                                nc.tensor.matmul(
                                    t,
                                    lhsT=xT_sb[:, k, st * P : (st + 1) * P],
                                    rhs=wT_sb[:, k, 2 * D : 3 * D],
                                    start=(k == 0),
                                    stop=(k == KT - 1),
                                )
                    for g, t in zip(blk, tiles):
                        if g[0] == "qk":
                            _, ft, qc = g
                            # copy out with per-partition bias + Q prescale
                            nc.vector.tensor_scalar(
                                out=qkT[:, ft, qc * 512 : (qc + 1) * 512],
                                in0=t,
                                scalar1=qkb_sb[:, ft : ft + 1],
                                scalar2=(sQ if ft < FT_QK // 2 else 1.0),
                                op0=mybir.AluOpType.add,
                                op1=mybir.AluOpType.mult,
                            )
                        else:
                            _, st, _ = g
                            nc.vector.tensor_tensor(
                                out=v_sb[:, st, :, 0:HD],
                                in0=t.rearrange("p (h d) -> p h d", h=H),
                                in1=vb_sb.rearrange("p (h d) -> p h d", h=H),
                                op=mybir.AluOpType.add,
                            )

            def emit_e(gbt):
                # E = exp(a*sigmoid(gb)) = exp(a/2*tanh(gb/2) + a/2): tanh
                # shares the exp table set (sigmoid does not), so the whole
                # kernel uses ONE ACT table — no per-rep set switches. The
                # tanh writes straight into the double-buffered e_sb (gbt
                # frees after this first pass, unblocking the next rep's
                # graph_bias DMA), then exp runs in place. All tanhs before
                # all exps to avoid ACT-FIFO head-of-line blocking.
                e_sb = work.tile([P, NT, N], F16, tag="e_sb", bufs=2,
                                 name="e_sb")
                for t4 in range(4):
                    nc.scalar.activation(
                        out=e_sb[:, 2 * t4 : 2 * t4 + 2, :],
                        in_=gbt[:, 2 * t4 : 2 * t4 + 2, :],
                        func=mybir.ActivationFunctionType.Tanh,
                        scale=0.5,
                    )
                for t4 in range(4):
                    sl = e_sb[:, 2 * t4 : 2 * t4 + 2, :]
                    nc.scalar.activation(out=sl, in_=sl, func=Exp,
                                         scale=0.5 * ten_a, bias=b05)
                return e_sb

            def phase2(e_sb):
                # ---- phase 2: attention, (qc, head-pair) groups ----
                for qc in range(QC):
                    for hp in range(4):
                        pvab = ps_pv.tile(
                            [HD + 1, N], F32, tag="pv", bufs=1, name="pvab"
                        )
                        for kt in range(NT):
                            sT = ps_s.tile([P, N], F32, tag="sT", bufs=2,
                                           name="sT")
                            for sub in range(2):
                                qp = 64 * sub
                                nc.tensor.matmul(
                                    sT[:, sub * 512 : (sub + 1) * 512],
                                    lhsT=qkT[
                                        qp : qp + HD,
                                        FT_QK // 2 + hp,
                                        kt * P : (kt + 1) * P,
                                    ],
                                    rhs=qkT[
                                        qp : qp + HD, hp,
                                        qc * 512 : (qc + 1) * 512,
                                    ],
                                    start=True,
                                    stop=True,
                                )
                            es = work.tile([P, N], F16, tag="es", bufs=4,
                                           name="es")
                            nc.scalar.activation(
                                out=es, in_=sT, func=Exp, scale=ten_a,
                                bias=neg5,
                            )
                            eT = work.tile([P, N], F16, tag="eT", bufs=4,
                                           name="eT")
                            esl = e_sb[:, kt, qc * 512 : (qc + 1) * 512]
                            for sub in range(2):
                                nc.vector.tensor_tensor(
                                    out=eT[:, sub * 512 : (sub + 1) * 512],
                                    in0=es[:, sub * 512 : (sub + 1) * 512],
                                    in1=esl,
                                    op=mybir.AluOpType.mult,
                                )
                            for sub in range(2):
                                nc.tensor.matmul(
                                    pvab[:, sub * 512 : (sub + 1) * 512],
                                    lhsT=v_sb[:, kt, 2 * hp + sub, :],
                                    rhs=eT[:, sub * 512 : (sub + 1) * 512],
                                    start=(kt == 0),
                                    stop=(kt == NT - 1),
                                )
                        recip = work.tile([1, N], F32, tag="recip", bufs=1,
                                          name="recip")
                        nc.vector.reciprocal(recip, pvab[HD : HD + 1, :])
                        bc = work.tile([HD, N], F32, tag="bc", bufs=1,
                                       name="bc")
                        nc.gpsimd.partition_broadcast(bc, recip)
                        for sub in range(2):
                            qp = 64 * sub
                            nc.vector.tensor_tensor(
                                out=attnT[
                                    qp : qp + HD, hp,
                                    qc * 512 : (qc + 1) * 512,
                                ],
                                in0=pvab[0:HD, sub * 512 : (sub + 1) * 512],
                                in1=bc[:, sub * 512 : (sub + 1) * 512],
                                op=mybir.AluOpType.mult,
                            )
                    # ---- phase 3 for qc0's seq tiles (qc1's are deferred
                    # past the next rep's first projections) ----
                    if qc == 0:
                        for st in range(4):
                            phase3(st)

            # ---- software-pipelined rep loop ----
            xT_sb, gbt = input_dmas()
            for rep in range(reps):
                proj_groups(groups_a, xT_sb)        # pair-0 Q/K first
                if rep > 0:
                    for st in range(4, NT):         # prev rep's phase-3 tail
                        phase3(st)
                proj_groups(groups_b, xT_sb)
                e_sb = emit_e(gbt)
                if rep + 1 < reps:
                    xT_sb, gbt = input_dmas()       # prefetch next rep
                phase2(e_sb)
            for st in range(4, NT):
                phase3(st)
    nc.finalize()
    return nc


def kernel(x, graph_bias, in_proj_w, in_proj_b, out_proj_w, out_proj_b,
           bias_strength):
    x = np.asarray(x, dtype=np.float32)
    graph_bias = np.ascontiguousarray(np.asarray(graph_bias, dtype=np.float32))
    in_proj_w = np.asarray(in_proj_w, dtype=np.float32)
    in_proj_b = np.asarray(in_proj_b, dtype=np.float32)
    out_proj_w = np.asarray(out_proj_w, dtype=np.float32)
    out_proj_b = np.asarray(out_proj_b, dtype=np.float32)
    alpha = 1.0 / (1.0 + np.exp(-float(np.asarray(bias_strength))))
    ten_a = 10.0 * alpha

    key = round(ten_a, 9)
    if key not in _CACHE:
        _CACHE[key] = build_nc(ten_a)
    nc = _CACHE[key]

    wT = np.ascontiguousarray(in_proj_w.T)          # [512, 1536]
    woT = np.ascontiguousarray(out_proj_w.T)        # [512, 512]
    qkb = np.ascontiguousarray(
        in_proj_b[: 2 * D].reshape(FT_QK, P).T      # [128, 8]
    )
    vb = in_proj_b[2 * D :]
    bo = out_proj_b

    in_maps = []
    for b in range(B):
        in_maps.append({
            "xT": np.ascontiguousarray(x[b].T),
            "wT": wT,
            "woT": woT,
            "gb": graph_bias[b],
            "qkb": qkb,
            "vb": vb,
            "bo": bo.reshape(1, D),
            "ones1": np.ones((1, P), dtype=np.float32),
        })

    global _saved_in_maps
    _saved_in_maps = in_maps
    res = run_bass_kernel_spmd(nc, in_maps, core_ids=list(range(B)))
    out = np.stack([res.results[b]["y"] for b in range(B)], axis=0)
    return out.astype(np.float32)
